# revision 32
# baseline (speedup 1.0000x reference)
"""Trainium2 Bass kernel for a causal single-head attention block.

reference:
    K = x @ Wk; Q = x @ Wq; V = x @ Wv          # x [B,T,C], W [C,H]
    scores = (Q @ K^T) * C**-0.5, causal masked
    out = softmax(scores) @ V                    # [B,T,H]

B=512, T=256, C=384, H=64. Pure data parallel over batch across 8 cores
(64 batches per core); the three projection weights are replicated.

Host side: x is pre-transposed/cast to a device-contiguous bf16 layout
[n_chunks, c(128), ch, ct, t] (x is read once anyway, so this rides along
with the mandatory host->device copy); out returns in the device-natural
layout [pair_idx, p(128), pair, tt, h] and is un-permuted on the host.

Device dataflow per pair of batches (all stages pair-granular):
    a = [0|Wk]^T @ x^T  -> psum a [128, 2, T]: rows 64:128 = K^T
    b = [Wv|Wq]^T @ x^T -> psum b [128, 2, T]: rows 0:64 = V^T, 64:128 = Q^T
    (K^T and Q^T both at partition base 64: matmul lhsT/rhs must start at
    the same SB partition index.)
    V [t, h] via PE transpose of V^T; copied next to a preset ones column
    scoresT = K @ Q^T   (transposed scores [s, t]; the fully-masked
          s-tile-1/t-tile-0 block is never computed)
    expT = exp(scoresT * SCALE)  - one ACT op per pair
    causal mask: one strided affine_select per pair on the two diagonal
          128x128 blocks of each batch (Pool engine)
    out_unnorm[t,0:64], denom[t] = expT^T @ [V | 1]  (ones column folds the
          softmax denominator into the PV matmul)
    out[t,h] = out_unnorm * (1/denom)  (DVE reciprocal + broadcast mul)

Schedule: a 3-deep software pipeline at pair granularity keeps the PE
instruction stream free of cross-engine waits (which would also drop the
PE out of its 2.4GHz p-state):
    iteration p:  proj(p) | scores+exp+mask(p-1) | PV+store(p-2)
PV(p-2) only waits on exp/mask finished a whole iteration earlier, and
scores(p-1) operands were copied to SBUF during iteration p-1.
"""

import os
import sys

for _p in ("/opt/trn_rl_repo", "/root/.axon_site/_ro/trn_rl_repo"):
    if os.path.isdir(_p) and _p not in sys.path:
        sys.path.append(_p)

from contextlib import ExitStack

import ml_dtypes
import numpy as np

import concourse.bass as bass
import concourse.tile as tile
from concourse import bacc, mybir
from concourse.bass_utils import run_bass_kernel_spmd
from concourse.masks import make_identity

B, T, C, H = 512, 256, 384, 64
N_CORES = 8
BPC = B // N_CORES  # batches per core
SCALE = float(C) ** -0.5
NCT = C // 128  # contraction tiles for the projections

F32 = mybir.dt.float32


class Cfg:
    cdt = mybir.dt.bfloat16  # compute dtype on the PE array
    np_cdt = ml_dtypes.bfloat16
    pair = 2  # batches per pipeline stage (psum bank limit: pair*T <= 512)
    chunk = 8  # batches per x-load DMA
    acopy_act = True  # K^T psum->sbuf copy on ACT (else DVE)
    vones_pool = False  # V psum->sbuf copy on Pool — dead end: Pool can't read PSUM
    aps_bufs = 2
    bps_bufs = 2
    scps_bufs = 1
    ops_bufs = 1
    vps_bufs = 1
    stage = "full"  # timing decomposition: "mm" | "proj" | "sc" | "full"
    split_xdma = False  # split x chunk loads across the two HWDGE rings (hurts)
    pvl = 2  # extra iterations of slack between scores and PV stages
    unroll = 1  # python-unrolled body repeats inside the For_i (timing only)
    loop_r = 0  # if >1, wrap the whole body in a For_i repeat loop (timing)


def build_body(ctx, tc, out, xT, wab, n_b, cfg):
    nc = tc.nc
    cdt = cfg.cdt
    pair = cfg.pair
    n_pairs = n_b // pair
    cpp = cfg.chunk // pair  # pairs per DMA chunk

    consts = ctx.enter_context(tc.tile_pool(name="consts", bufs=1))
    xpool = ctx.enter_context(tc.tile_pool(name="x", bufs=3))
    kqpool = ctx.enter_context(tc.tile_pool(name="kq", bufs=6))
    epool = ctx.enter_context(tc.tile_pool(name="exp", bufs=4))
    opool = ctx.enter_context(tc.tile_pool(name="o", bufs=4))
    spool = ctx.enter_context(tc.tile_pool(name="small", bufs=4))
    psum = ctx.enter_context(tc.tile_pool(name="ps", bufs=1, space="PSUM"))

    # --- constants ---------------------------------------------------------
    # wab [C, 4H] = [0|Wk|Wv|Wq]: a-lhsT = cols 0:128, b-lhsT = cols 128:256
    # (zeros so scores can use full-128-partition operands -> FWL weight loads)
    wab_sb = consts.tile([128, NCT, 4 * H], cdt)
    nc.sync.dma_start(out=wab_sb, in_=wab.rearrange("(ct c) m -> c ct m", c=128))
    ident64 = consts.tile([64, 64], cdt)
    make_identity(nc, ident64)
    vones_slots = []
    for i in range(4):
        vs = consts.tile([128, pair, 2, H + 1], cdt, name=f"vones{i}", tag=f"vones{i}")
        nc.gpsimd.memset(vs[:, :, :, H : H + 1], 1.0)
        vones_slots.append(vs)

    def body(iv=None):
        xg_tiles = {}

        def load_chunk(c):
            if c * cpp >= n_pairs:
                return
            xgc = xpool.tile([128, cfg.chunk, NCT, T], cdt, tag="xg")
            xg_tiles[c] = xgc
            if cfg.split_xdma:
                # one ring per half-chunk: both HWDGE rings pull concurrently
                h = cfg.chunk // 2
                nc.sync.dma_start(out=xgc[:, 0:h], in_=xT[c, :, 0:h])
                nc.scalar.dma_start(out=xgc[:, h:], in_=xT[c, :, h:])
            else:
                nc.sync.dma_start(out=xgc, in_=xT[c])

        def s_proj(p):
            c = p // cpp
            if p % cpp == 0:
                load_chunk(c + 1)  # prefetch one chunk ahead
            xgc = xg_tiles[c]
            xoff = (p % cpp) * pair
            a_ps = psum.tile([128, pair, T], F32, tag="aps", bufs=cfg.aps_bufs)
            b_ps = psum.tile([128, pair, T], F32, tag="bps", bufs=cfg.bps_bufs)
            for ct in range(NCT):
                nc.tensor.matmul(
                    a_ps,
                    wab_sb[:, ct, 0:128],
                    xgc[:, xoff : xoff + pair, ct, :],
                    start=(ct == 0),
                    stop=(ct == NCT - 1),
                )
            for ct in range(NCT):
                nc.tensor.matmul(
                    b_ps,
                    wab_sb[:, ct, 128:256],
                    xgc[:, xoff : xoff + pair, ct, :],
                    start=(ct == 0),
                    stop=(ct == NCT - 1),
                )
            if cfg.stage == "mm":
                return None, None
            a_sb = kqpool.tile([128, pair, T], cdt, tag="asb")
            if cfg.acopy_act:
                nc.scalar.copy(a_sb[64:128], a_ps[64:128])
            else:
                nc.vector.tensor_copy(a_sb[64:128], a_ps[64:128])
            b_sb = kqpool.tile([128, pair, T], cdt, tag="bsb")
            nc.vector.tensor_copy(b_sb, b_ps)
            return a_sb, b_sb

        def s_sc(p, a_sb, b_sb):
            # V natural [t, h] for the pair via PE transposes of V^T
            v_ps = psum.tile([128, pair, 2, H], cdt, tag="vps", bufs=cfg.vps_bufs)
            for j in range(pair):
                for tt in range(2):
                    nc.tensor.transpose(
                        v_ps[:, j, tt, :],
                        b_sb[0:64, j, tt * 128 : (tt + 1) * 128],
                        ident64,
                    )
            vones_p = vones_slots[p % 4]
            if cfg.vones_pool:
                nc.gpsimd.tensor_copy(vones_p[:, :, :, 0:H], v_ps)
            else:
                nc.vector.tensor_copy(vones_p[:, :, :, 0:H], v_ps)
            # transposed scores for the pair in one 2-bank psum tile (512
            # padding keeps each batch bank-aligned):
            # per batch, cols 0:T = s-tile 0 (all t), T:T+128 = s-tile 1
            sc_pp = psum.tile([128, pair, 512], F32, tag="scps", bufs=cfg.scps_bufs)
            for j in range(pair):
                nc.tensor.matmul(
                    sc_pp[:, j, 0:T],
                    a_sb[64:128, j, 0:128],
                    b_sb[64:128, j, :],
                    start=True,
                    stop=True,
                )
                nc.tensor.matmul(
                    sc_pp[:, j, T : T + 128],
                    a_sb[64:128, j, 128:T],
                    b_sb[64:128, j, 128:T],
                    start=True,
                    stop=True,
                )
            expT_p = epool.tile([128, pair, T + 128], cdt, tag="expT")
            nc.scalar.activation(
                expT_p,
                sc_pp[:, :, 0 : T + 128],
                mybir.ActivationFunctionType.Exp,
                scale=SCALE,
            )
            # causal mask on the diagonal blocks (cols 0:128 and 256:384 of
            # each batch): keep where -s + t >= 0. One strided op per batch
            # (the affine_select ISA pattern allows at most 2 free dims).
            for j in range(pair):
                blocks = expT_p[:, j].rearrange("p (n c) -> p n c", c=128)[:, 0::2, :]
                nc.gpsimd.affine_select(
                    out=blocks,
                    in_=blocks,
                    compare_op=mybir.AluOpType.is_ge,
                    fill=0.0,
                    base=0,
                    pattern=[[0, 2], [1, 128]],
                    channel_multiplier=-1,
                )
            return expT_p, vones_p

        def s_pv(p, expT_p, vones_p):
            o_ps = psum.tile([128, pair, 2, H + 1], F32, tag="ops", bufs=cfg.ops_bufs)
            for j in range(pair):
                expT = expT_p[:, j, :]
                vones = vones_p[:, j, :, :]
                # PV + folded denominator: out_unnorm = expT^T @ [V | 1]
                nc.tensor.matmul(
                    o_ps[:, j, 0, :],
                    expT[:, 0:128],
                    vones[:, 0, :],
                    start=True,
                    stop=True,
                )
                nc.tensor.matmul(
                    o_ps[:, j, 1, :],
                    expT[:, 128:T],
                    vones[:, 0, :],
                    start=True,
                    stop=False,
                )
                nc.tensor.matmul(
                    o_ps[:, j, 1, :],
                    expT[:, T : T + 128],
                    vones[:, 1, :],
                    start=False,
                    stop=True,
                )
            recip = spool.tile([128, pair, 2, 1], F32)
            nc.vector.reciprocal(recip, o_ps[:, :, :, H : H + 1])
            rbc = bass.AP(
                tensor=recip.tensor,
                offset=recip.offset,
                ap=[recip.ap[0], recip.ap[1], recip.ap[2], [0, H]],
            )
            out_sb = opool.tile([128, pair, 2, H], cdt, tag="osb")
            nc.vector.tensor_mul(out_sb, o_ps[:, :, :, 0:H], rbc)
            nc.scalar.dma_start(out=out[p], in_=out_sb)

        # Stage issue order per iteration: scores(p-1), PV(p-1-pvl), proj(p).
        # scores first keeps exp at the head of ACT's queue (not blocked
        # behind the a-copy), and the PE starts every iteration with
        # instructions whose inputs finished a full iteration earlier.
        load_chunk(0)
        ab = {}
        st = {}
        attn = cfg.stage in ("sc", "full")
        for p in range(n_pairs + 1 + cfg.pvl + 1):
            if attn and 0 <= p - 1 < n_pairs:
                st[p - 1] = s_sc(p - 1, *ab.pop(p - 1))
            if cfg.stage == "full" and 0 <= p - 1 - cfg.pvl < n_pairs:
                s_pv(p - 1 - cfg.pvl, *st.pop(p - 1 - cfg.pvl))
            if p < n_pairs:
                ab[p] = s_proj(p)

    if cfg.loop_r and cfg.loop_r > 1:
        with tc.For_i(0, cfg.loop_r, 1) as iv:
            for _ in range(cfg.unroll):
                body(iv)
    else:
        for _ in range(cfg.unroll):
            body()


def build_kernel(n_b=BPC, cfg=None):
    cfg = cfg or Cfg()
    nc = bacc.Bacc("TRN2", target_bir_lowering=False, debug=False)
    # xT: host pre-arranged [n_ch, c(128), ch, ct, t] so chunk loads are one
    # fully-contiguous DMA; out: device-natural [pair, p(128), 2, tt, h] that
    # the host un-permutes after the gather.
    xT = nc.dram_tensor(
        "xT", [n_b // cfg.chunk, 128, cfg.chunk, NCT, T], cfg.cdt,
        kind="ExternalInput",
    ).ap()
    wab = nc.dram_tensor("wab", [C, 4 * H], cfg.cdt, kind="ExternalInput").ap()
    out = nc.dram_tensor(
        "out", [n_b // cfg.pair, 128, cfg.pair, 2, H], cfg.cdt,
        kind="ExternalOutput",
    ).ap()

    with tile.TileContext(nc) as tc, ExitStack() as ctx:
        build_body(ctx, tc, out, xT, wab, n_b, cfg)
    nc.compile()
    return nc


def prep_inputs(x, Wk, Wq, Wv, n_cores=N_CORES, cfg=None):
    """Shard over batch + host-side pre-transpose/cast of x."""
    cfg = cfg or Cfg()
    x = np.asarray(x, dtype=np.float32)
    Wk = np.asarray(Wk, dtype=np.float32)
    Wq = np.asarray(Wq, dtype=np.float32)
    Wv = np.asarray(Wv, dtype=np.float32)
    bpc = x.shape[0] // n_cores
    ch = cfg.chunk
    wab = np.concatenate([np.zeros_like(Wk), Wk, Wv, Wq], axis=1).astype(cfg.np_cdt)
    in_maps = []
    for i in range(n_cores):
        shard = x[i * bpc : (i + 1) * bpc]  # [bpc, T, C]
        xh = shard.reshape(bpc // ch, ch, T, NCT, 128).transpose(0, 4, 1, 3, 2)
        in_maps.append({"xT": np.ascontiguousarray(xh).astype(cfg.np_cdt), "wab": wab})
    return in_maps


_NC_CACHE = {}


def kernel(x, Wk, Wq, Wv):
    cfg = Cfg()
    key = (x.shape[0] // N_CORES, cfg.cdt, cfg.pair, cfg.chunk)
    if key not in _NC_CACHE:
        _NC_CACHE[key] = build_kernel(n_b=key[0], cfg=cfg)
    nc = _NC_CACHE[key]
    in_maps = prep_inputs(x, Wk, Wq, Wv, cfg=cfg)
    res = run_bass_kernel_spmd(nc, in_maps, list(range(N_CORES)))
    shards = []
    for r in res.results:
        o = r["out"]  # [n_pairs, 128, pair, 2, H] device-natural
        shards.append(o.transpose(0, 2, 3, 1, 4).reshape(BPC, T, H))
    full = np.concatenate(shards, axis=0)
    return np.ascontiguousarray(full.astype(np.float32))
